# revision 46
# baseline (speedup 1.0000x reference)
"""GAT-style aggregation kernel for Trainium2, 8 NeuronCores.

Math (per graph):
  h = X @ W;  s1 = X @ c1;  s2 = X @ c2   (c = W b, folded)
  P[i,j] = exp(leaky_relu(s1_i + s2_j, 0.2))
         = u_i*a_j + relu(v_i*b_j - u_i*a_j)          (exact identity)
    where u=e^{s1}, v=e^{0.2 s1}, a=e^{s2}, b=e^{0.2 s2}
  l_i = sum_j P[i,j] = u_i*A + R1_i   (A = sum_j a_j, R1 = rowsum of relu term)
  r_i = 1/l_i
  w_j = sum_i r_i P[i,j] = Q*a_j + sum_i r_i E[i,j]   (Q = sum_i u_i r_i)
  out = elu(sum_j w_j h[j,:]) = elu(v2^T W),  v2 = Q*(X^T a) + X^T w_relu

Sharding: batch dim 16 -> 2 graphs per core, W/b replicated, gather on host.

Engine plan (per core):
  gpsimd : s1/s2 column vectors via mult+row-accum against replicated c rows
  scalar : exps (column layout), half of the N^2 relu+rowsum, psum copies
  vector : other half of the N^2 relu+rowsum, small ops
  tensor : rank-2 score matmuls (4 row-groups), r-weighted column sums
           (4-way column tiling), tail contractions
"""

import numpy as np
from contextlib import ExitStack

B_FULL = 16
N_CORES = 8
B_LOC = B_FULL // N_CORES  # 2
N = 2048
F = 128
NBLK = N // 128  # 16

_CACHE = {}


def _build():
    import concourse.bass as bass
    import concourse.tile as tile
    from concourse import bacc, mybir
    from concourse.masks import make_identity

    f32 = mybir.dt.float32
    f32r = mybir.dt.float32r
    AF = mybir.ActivationFunctionType
    ALU = mybir.AluOpType

    nc = bacc.Bacc("TRN2", target_bir_lowering=False, debug=False)
    x = nc.dram_tensor("x_local", [B_LOC, N, F], mybir.dt.float32r,
                       kind="ExternalInput").ap()
    w_in = nc.dram_tensor("w_in", [F, F], f32, kind="ExternalInput").ap()
    b_in = nc.dram_tensor("b_in", [2 * F, 1], f32, kind="ExternalInput").ap()
    out = nc.dram_tensor("out_local", [B_LOC, F], f32, kind="ExternalOutput").ap()
    wscr = nc.dram_tensor("wscratch", [B_LOC, N], mybir.dt.float32r,
                          kind="Internal").ap()

    # row-group base partition for score matmuls: graph g, block parity p
    ROWBASE = {(0, 0): 0, (0, 1): 64, (1, 0): 32, (1, 1): 96}

    with tile.TileContext(nc) as tc, ExitStack() as ctx:
        singles = ctx.enter_context(tc.tile_pool(name="singles", bufs=1))
        sb_e = ctx.enter_context(tc.tile_pool(name="sb_e", bufs=64))
        sb_small = ctx.enter_context(tc.tile_pool(name="sb_small", bufs=8))
        # PSUM: one pool, 4 slots x [128,1024] f32 (2 banks each) = 8 banks.
        # Phase A uses all slots as score-tile buffers; phase B's w quarter
        # accumulators and ph1/tail tiles share the same ring.
        ps_d = ctx.enter_context(tc.tile_pool(name="ps_d", bufs=4, space="PSUM"))

        # ---------------- setup ----------------
        bf16 = mybir.dt.bfloat16
        ident_f = sb_small.tile([128, 128], f32, tag="ident_f")
        make_identity(nc, ident_f)
        identity = singles.tile([128, 128], f32r, tag="identity")
        nc.vector.tensor_copy(identity, ident_f)
        ones_row = singles.tile([1, 128], f32, tag="ones_row")
        nc.vector.memset(ones_row, 1.0)
        nones_row = singles.tile([1, 128], f32, tag="nones_row")
        nc.vector.memset(nones_row, -1.0)
        ones_col = singles.tile([128, 1], f32, tag="ones_col")
        nc.vector.memset(ones_col, 1.0)

        w_nat = singles.tile([128, 128], f32, tag="w_nat")  # [k, f]
        nc.sync.dma_start(w_nat, w_in)
        bcols = singles.tile([128, 2], f32, tag="bcols")  # [b2 | b1]
        nc.sync.dma_start(bcols[:, 0:1], b_in[128:256, :])
        nc.sync.dma_start(bcols[:, 1:2], b_in[0:128, :])

        # c cols: c4b = [c2 | c1] in bf16 (c_t = W b_t), for the s matmuls
        # against the bf16 transposed X. Needs W^T as lhsT.
        wt_ps = ps_d.tile([128, 128], f32, tag="d")
        nc.tensor.transpose(wt_ps, w_nat, ident_f)
        wt = singles.tile([128, 128], f32, tag="wt")
        nc.scalar.copy(wt, wt_ps)
        c4_ps = ps_d.tile([128, 2], f32, tag="d")
        nc.tensor.matmul(c4_ps, lhsT=wt, rhs=bcols, start=True, stop=True)
        c4b = singles.tile([128, 2], bf16, tag="c4b")
        nc.scalar.copy(c4b, c4_ps)

        # per-graph persistent tiles
        xr = singles.tile([128, B_LOC, NBLK, 128], f32r, tag="xr")  # [p, g, blk, k]
        xtb = singles.tile([128, B_LOC, NBLK, 128], bf16, tag="xtb")  # [k, g, blk, n]
        # score-vector rows, replicated at the 2 row-group bases per graph
        vu_rows = singles.tile([98, 2048], bf16, tag="vu_rows")  # [v; u]
        ba_rows = singles.tile([98, 2048], bf16, tag="ba_rows")  # [b; -a]
        scol = {}   # [128, 2, 16] s1/s2 column layout
        stk = {}    # [128, 4, 16] exp'd columns (v, u, b, -a)
        a_col, u_slice, uA, rc, Ac, nQc = {}, {}, {}, {}, {}, {}

        def ph1(g):
            # x load in 4 chunks so downstream work starts early
            xg = x[g].rearrange("(blk p) k -> p blk k", p=128)
            for (c0, cn) in ((0, 2), (2, 2), (4, 4), (8, 8)):
                nc.sync.dma_start(xr[:, g, c0:c0 + cn, :],
                                  xg[:, c0:c0 + cn, :])
            # transpose X blocks (pairs) to bf16 X^T; copies split Act/DVE
            for p2 in range(8):
                xt_ps = ps_d.tile([128, 2, 128], f32r, tag="d")
                for j in range(2):
                    b = 2 * p2 + j
                    nc.tensor.transpose(xt_ps[:, j, :], xr[:, g, b, :],
                                        identity)
                if p2 % 2 == 0:
                    nc.scalar.copy(xtb[:, g, 2 * p2:2 * p2 + 2, :], xt_ps)
                else:
                    nc.vector.tensor_copy(xtb[:, g, 2 * p2:2 * p2 + 2, :],
                                          xt_ps)
            # s columns via bf16 matmuls (FWL weight loads):
            # sc[n, (b), t] with t: 0 = s2, 1 = s1
            sc_ps = ps_d.tile([128, NBLK, 2], f32, tag="d")
            for b in range(NBLK):
                nc.tensor.matmul(sc_ps[:, b, :], lhsT=xtb[:, g, b, :],
                                 rhs=c4b, start=True, stop=True,
                                 skip_group_check=True)

            # exp in column layout: stack [128, (v,u,b,na), 16]
            st = singles.tile([128, 4, NBLK], f32r, tag=f"stk{g}")
            nc.scalar.activation(st[:, 0, :], sc_ps[:, :, 1], AF.Exp,
                                 scale=0.2)
            nc.scalar.activation(st[:, 1, :], sc_ps[:, :, 1], AF.Exp)
            nc.scalar.activation(st[:, 2, :], sc_ps[:, :, 0], AF.Exp,
                                 scale=0.2)
            ac = singles.tile([128, NBLK], f32r, tag=f"acol{g}")
            apart = singles.tile([128, 1], f32, tag=f"apart{g}")
            nc.scalar.activation(ac, sc_ps[:, :, 0], AF.Exp, accum_out=apart)
            nc.vector.tensor_scalar(st[:, 3, :], ac, -1.0, None, ALU.mult)
            stk[g] = st
            a_col[g] = ac
            u_slice[g] = st[:, 1, :]

            # A = sum_j a_j -> [128,1] broadcast
            A1_ps = ps_d.tile([1, 1], f32, tag="d")
            nc.tensor.matmul(A1_ps, lhsT=ones_col, rhs=apart,
                             start=True, stop=True)
            A1 = singles.tile([1, 1], f32, tag=f"A1{g}")
            nc.scalar.copy(A1, A1_ps)
            Ac_ps = ps_d.tile([128, 1], f32, tag="d")
            nc.tensor.matmul(Ac_ps, lhsT=ones_row, rhs=A1, start=True, stop=True)
            Acg = singles.tile([128, 1], f32, tag=f"Ac{g}")
            nc.vector.tensor_copy(Acg, Ac_ps)
            Ac[g] = Acg

            # transpose exp'd columns to rows: [128, 4*16] -> [64, 128]
            t_ps = ps_d.tile([64, 128], f32r, tag="d")
            nc.tensor.transpose(t_ps, st.rearrange("p t b -> p (t b)"), identity)
            t_sb = sb_small.tile([64, 128], bf16, tag=f"t64{g}")
            nc.vector.tensor_copy(t_sb, t_ps)
            # flatten partition-blocks to [1, 2048] rows at both row bases
            for pos in range(2):
                base = ROWBASE[(g, pos)]
                nc.sync.dma_start(vu_rows[base:base + 1, :], t_sb[0:16, :])
                nc.sync.dma_start(vu_rows[base + 1:base + 2, :], t_sb[16:32, :])
                nc.sync.dma_start(ba_rows[base:base + 1, :], t_sb[32:48, :])
                nc.sync.dma_start(ba_rows[base + 1:base + 2, :], t_sb[48:64, :])

            # uA = u * A
            uAg = singles.tile([128, NBLK], f32, tag=f"uA{g}")
            nc.vector.tensor_scalar(uAg, st[:, 1, :], Acg, None, ALU.mult)
            uA[g] = uAg
            rcg = singles.tile([128, NBLK], f32r, tag=f"rc{g}")
            rc[g] = rcg

        # ---------------- main N^2 phase, split in two passes ----------------
        # Phase A: generate the full relu'd score matrix E (bf16) for one
        #   graph with a 4-deep PSUM pipeline; rowsums feed l -> r.
        # Phase B: 64 back-to-back M=1 matmuls sweep r^T E into w quarters.
        E_TILES = {}   # (g) -> list of 32 [128,1024] bf16 tiles
        W_SB = {}
        rcb = {}

        LT = {}

        def phaseA(g, blocks=None, fin=True):
            if g not in LT:
                ltg = singles.tile([128, NBLK], f32, tag=f"lt{g}")
                LT[g] = ltg
                E_TILES[g] = []
            ltg = LT[g]
            ets = E_TILES[g]
            for b in (blocks if blocks is not None else range(NBLK)):
                # interleave the two row-group halves' matmuls so they run
                # concurrently on the PE array
                d_tiles = []
                for h in range(2):
                    d_ps = ps_d.tile([128, 1024], f32, tag="d")
                    d_tiles.append(d_ps)
                for q in range(2):
                    for h in range(2):
                        base = ROWBASE[(g, h)]
                        nc.tensor.matmul(
                            d_tiles[h][:, 512 * q:512 * q + 512],
                            lhsT=vu_rows[base:base + 2, b * 128:(b + 1) * 128],
                            rhs=ba_rows[base:base + 2,
                                        1024 * h + 512 * q:
                                        1024 * h + 512 * q + 512],
                            start=True, stop=True,
                            tile_position=(base, 0),
                            skip_group_check=True)
                r1s = []
                for h in range(2):
                    et = sb_e.tile([128, 1024], bf16, tag="e")
                    r1 = sb_small.tile([128, 1], f32, tag=f"r1_{h}")
                    if (2 * b + h) % 2 == 0 and (2 * b + h) < 28:
                        nc.scalar.activation(et, d_tiles[h], AF.Relu,
                                             accum_out=r1)
                    else:
                        nc.vector.tensor_scalar(et, d_tiles[h], 0.0, 0.0,
                                                ALU.max, ALU.add,
                                                accum_out=r1)
                    r1s.append(r1)
                    ets.append(et)
                # l = uA + r1a + r1b
                nc.vector.scalar_tensor_tensor(
                    out=ltg[:, b:b + 1], in0=r1s[0],
                    scalar=uA[g][:, b:b + 1], in1=r1s[1],
                    op0=ALU.add, op1=ALU.add)
            if fin:
                with nc.allow_low_precision(reason="fp32r r for PE"):
                    nc.vector.reciprocal(rc[g], ltg)
                rb = singles.tile([128, NBLK], bf16, tag=f"rcb{g}")
                nc.vector.tensor_copy(rb, rc[g])
                rcb[g] = rb

        def phaseB(g, quarters=range(4)):
            # w quarter accumulators live one at a time in the psum ring
            ets = E_TILES[g]
            if g not in W_SB:
                wsb = singles.tile([1, 2048], f32r, tag=f"wsb{g}")
                W_SB[g] = wsb
            w_sb = W_SB[g]
            for q in quarters:
                w_ps = ps_d.tile([1, 512], f32, tag="d")
                for b in range(NBLK):
                    # E tile layout: per block, half h covers j-cols
                    # [1024h, 1024h+1024); quarter q is (h, qq) = divmod(q, 2)
                    h, qq = divmod(q, 2)
                    nc.tensor.matmul(
                        w_ps,
                        lhsT=rcb[g][:, b:b + 1],
                        rhs=ets[2 * b + h][:, 512 * qq:512 * qq + 512],
                        start=(b == 0), stop=(b == NBLK - 1),
                        skip_group_check=True)
                if q % 2 == 0:
                    nc.scalar.copy(w_sb[:, 512 * q:512 * q + 512], w_ps)
                else:
                    nc.vector.tensor_copy(w_sb[:, 512 * q:512 * q + 512], w_ps)

        # ---------------- tail ----------------
        AW = {}

        def tail_dma(g):
            # w row -> DRAM -> column layout inside aw
            w_sb = W_SB[g]
            nc.sync.dma_start(wscr[g:g + 1, :], w_sb)
            aw = singles.tile([128, NBLK, 2], f32r, tag=f"aw{g}")
            nc.vector.tensor_copy(
                aw[:, :, 0:1], a_col[g].rearrange("p (b o) -> p b o", o=1))
            nc.sync.dma_start(
                aw[:, :, 1:2],
                wscr[g:g + 1, :].rearrange("one (blk p) -> (one p) blk", p=128))
            AW[g] = aw

        def tail(g):
            aw = AW[g]

            # Q = sum_i u_i r_i -> negated broadcast
            ur = singles.tile([128, NBLK], f32, tag=f"ur{g}")
            nc.vector.tensor_mul(ur, u_slice[g], rc[g])
            qt_ps = ps_d.tile([1, NBLK], f32, tag="d")
            nc.tensor.matmul(qt_ps, lhsT=ones_col, rhs=ur, start=True, stop=True)
            qt_sb = singles.tile([1, NBLK], f32, tag=f"qtsb{g}")
            Q1 = singles.tile([1, 1], f32, tag=f"Q1{g}")
            nc.scalar.activation(qt_sb, qt_ps, AF.Identity, accum_out=Q1)
            nq_ps = ps_d.tile([128, 1], f32, tag="d")
            nc.tensor.matmul(nq_ps, lhsT=ones_row, rhs=Q1, start=True, stop=True)
            nQ = singles.tile([128, 1], f32, tag=f"nQc{g}")
            nc.vector.tensor_copy(nQ, nq_ps)

            # rows [(-X^T a); (X^T w)] accumulated over blocks
            avw_ps = ps_d.tile([2, 128], f32, tag="d")
            for b in range(NBLK):
                nc.tensor.matmul(avw_ps, lhsT=aw[:, b, :], rhs=xr[:, g, b, :],
                                 start=(b == 0), stop=(b == NBLK - 1),
                                 skip_group_check=True)
            avw_sb = singles.tile([2, 128], f32r, tag=f"avwsb{g}")
            nc.vector.tensor_copy(avw_sb, avw_ps)
            avt_ps = ps_d.tile([128, 2], f32r, tag="d")
            nc.tensor.transpose(avt_ps, avw_sb, identity[0:2, 0:2])
            avt = singles.tile([128, 2], f32, tag=f"avt{g}")
            nc.vector.tensor_copy(avt, avt_ps)
            v2 = singles.tile([128, 1], f32, tag=f"v2{g}")
            nc.vector.scalar_tensor_tensor(
                out=v2, in0=avt[:, 0:1], scalar=nQ, in1=avt[:, 1:2],
                op0=ALU.mult, op1=ALU.add)

            res_ps = ps_d.tile([1, 128], f32, tag="d")
            nc.tensor.matmul(res_ps, lhsT=v2, rhs=w_nat, start=True, stop=True)
            # elu: t=relu(x); z=min(x,0); out = (exp(z)-1)+t
            t = singles.tile([1, 128], f32, tag=f"t{g}")
            nc.scalar.activation(t, res_ps, AF.Relu)
            z = singles.tile([1, 128], f32, tag=f"z{g}")
            nc.vector.tensor_scalar(z, res_ps, 0.0, None, ALU.min)
            e1 = singles.tile([1, 128], f32, tag=f"e1{g}")
            nc.scalar.activation(e1, z, AF.Exp)
            res = singles.tile([1, 128], f32, tag=f"res{g}")
            nc.vector.scalar_tensor_tensor(
                out=res, in0=e1, scalar=-1.0, in1=t, op0=ALU.add, op1=ALU.add)
            nc.sync.dma_start(out[g:g + 1, :], res)

        for g in range(B_LOC):
            ph1(g)
        phaseA(0)
        for k in range(4):
            phaseA(1, blocks=range(4 * k, 4 * k + 4), fin=(k == 3))
            phaseB(0, quarters=[k])
        tail_dma(0)
        phaseB(1)
        tail_dma(1)
        tail(0)
        tail(1)

    nc.compile()
    return nc


def _ensure_ntff_hook():
    import sys, types
    try:
        import antenv.axon_hooks  # noqa: F401
        return
    except ImportError:
        pass
    mod = types.ModuleType("antenv.axon_hooks")
    _h = {"h": None}
    mod.set_axon_ntff_profile_hook = lambda h: _h.__setitem__("h", h)
    mod.get_axon_ntff_profile_hook = lambda: _h["h"]
    sys.modules["antenv.axon_hooks"] = mod
    from trn_agent_boot.trn_boot import _ntff_profile_via_ctypes
    hook = _ntff_profile_via_ctypes("/opt/axon/libaxon_pjrt.so")
    if hook is not None:
        mod.set_axon_ntff_profile_hook(hook)


def kernel(graphs_feature, W, b):
    graphs_feature = np.ascontiguousarray(graphs_feature, dtype=np.float32)
    W = np.ascontiguousarray(W, dtype=np.float32)
    b = np.ascontiguousarray(b, dtype=np.float32)

    if "nc" not in _CACHE:
        _CACHE["nc"] = _build()
    nc = _CACHE["nc"]

    from concourse.bass_utils import run_bass_kernel_spmd

    in_maps = []
    for c in range(N_CORES):
        in_maps.append({
            "x_local": np.ascontiguousarray(graphs_feature[c * B_LOC:(c + 1) * B_LOC]),
            "w_in": W,
            "b_in": b,
        })
    import os
    trace = bool(os.environ.get("KTRACE"))
    if trace:
        _ensure_ntff_hook()
    r = run_bass_kernel_spmd(nc, in_maps, core_ids=list(range(N_CORES)),
                             trace=trace)
    o = np.concatenate([r.results[c]["out_local"] for c in range(N_CORES)])
    if not np.isfinite(o).all() or np.abs(o).max() > 1e6:
        # transient device corruption observed rarely; one retry
        r = run_bass_kernel_spmd(nc, in_maps, core_ids=list(range(N_CORES)),
                                 trace=False)
    if trace and r.exec_time_ns is not None:
        print(f"HW exec time: {r.exec_time_ns} ns")
        _CACHE["exec_time_ns"] = r.exec_time_ns
        _CACHE["trace"] = r.instructions_and_trace
        _CACHE["profile_json"] = r.profile_json
    outs = [r.results[c]["out_local"] for c in range(N_CORES)]
    return np.concatenate(outs, axis=0).astype(np.float32)


if __name__ == "__main__":
    nc = _build()
    print("build OK")
